# revision 31
# baseline (speedup 1.0000x reference)
"""Trainium2 Bass kernel for a single-head BERT attention (B=8, S=2048, E=1024, H=64).

Sharding: data-parallel over batch — one batch element per NeuronCore (8 cores).
Weights replicated. No collectives.

Layout (all matmuls bf16 with fp32 PSUM accumulation):
  qkT  = [Wq|Wk]^T-projection: [128, S] (rows 0-63 = q^T, 64-127 = k^T)
  v    = natural [S, H] per 128-row tile + appended ones column (softmax denom)
  ptT  = exp(scoresT / 8) per t-tile, multiplied by the mask on mixed blocks
  ctx  = ptT.T @ [v|1], rows normalized by 1/denom

Schedule is s-major streaming over variable-width column slabs. x^T arrives
as one contiguous DMA per slab (host pre-packs the DRAM layout); widths grow
across the kernel (narrow slabs land early to start compute sooner, wide
slabs carry the triangular bulk of the score work so the PE never starves
on the HBM-bound input stream). Per slab: project q/k for those s columns,
project v for the slab's t-tiles, run all mm1 columns ending in the slab,
exp them on ScalarE, then finish the slab's output tiles (mm2 + normalize +
one grouped store). Dummy warm-up matmuls run during the runtime preamble /
input DMA so the PE HAM clock gate is released before real work starts.
"""

import numpy as np
import ml_dtypes

import concourse.bass as bass  # noqa: F401  (import registers bass machinery)
import concourse.bacc as bacc
import concourse.mybir as mybir
import concourse.tile as tile
from concourse.bass_utils import run_bass_kernel_spmd

BF16 = ml_dtypes.bfloat16
B, S, E, H = 8, 2048, 1024, 64
P = 128          # partitions / tile edge
NS = S // P      # 16 seq tiles
NE = E // P      # 8 embed chunks
SB = 512         # max matmul free dim (one fp32 PSUM bank)
SLAB_W = [384, 512, 512, 640]   # s-major slab widths (sum = S, each %128 == 0)
SLAB_S0 = [sum(SLAB_W[:k]) for k in range(len(SLAB_W))]
NSL = len(SLAB_W)
WARMUP_MMS = 13  # dummy matmuls to release the HAM clock gate early

_cache: dict = {}
last_results = None  # BassKernelResults of the most recent run (for test harness)


def _plan_from_mask(mask: np.ndarray):
    """Derive the static block plan from the actual mask input.

    Returns (ranges, mask_items, n_uniq, mm2_lists, maskT):
      ranges[j]     = (lo, hi) element range of s that t-tile j must compute (or None)
      mask_items    = ((j, i, slot)) 128x128 blocks needing an elementwise mask
                      multiply; slot indexes the deduped unique-block upload
      mm2_lists[i]  = t-tiles contributing to output s-tile i
    Valid for every batch element simultaneously (classifications reduced over batch).
    """
    m = np.asarray(mask, dtype=bool)
    mt = np.ascontiguousarray(m.transpose(0, 2, 1))  # [B, t, s]
    blocks = mt.reshape(B, NS, P, NS, P)
    any_ = blocks.any(axis=(2, 4))   # [B, tj, si]
    all_ = blocks.all(axis=(2, 4))
    nz = any_.any(axis=0)            # not all-zero in some batch -> must compute
    allone = all_.all(axis=0)        # all-ones in every batch -> no mask needed
    mixed = nz & ~allone

    ranges = []
    for j in range(NS):
        cols = np.nonzero(nz[j])[0]
        if len(cols) == 0:
            ranges.append(None)
            continue
        ranges.append((int(cols.min()) * P, (int(cols.max()) + 1) * P))

    uniq = {}
    mask_items = []
    for j in range(NS):
        for i in range(NS):
            if not mixed[j, i]:
                continue
            key = blocks[:, j, :, i, :].tobytes()
            slot = uniq.setdefault(key, len(uniq))
            mask_items.append((j, i, slot))
    mm2 = [tuple(int(j) for j in np.nonzero(nz[:, i])[0]) for i in range(NS)]
    return ranges, tuple(mask_items), len(uniq), mm2, mt


def _build_nc(ranges, mask_items, n_uniq, mm2, has_bqk, has_bv):
    dt = mybir.dt
    n_mb = max(n_uniq, 1)
    nc = bacc.Bacc("TRN2", target_bir_lowering=False, debug=False, num_devices=8)

    # x^T packed host-side as slab-major [p][slab: chunk-major cols] so each
    # slab is a single contiguous read per partition.
    xq_d = nc.dram_tensor("xq", [P, NE * S], dt.bfloat16, kind="ExternalInput").ap()
    wqk_d = nc.dram_tensor("wqk", [P, NE * 2 * H], dt.bfloat16, kind="ExternalInput").ap()
    wv_d = nc.dram_tensor("wv", [P, NE * H], dt.bfloat16, kind="ExternalInput").ap()
    mb_d = nc.dram_tensor("maskb", [P, n_mb * P], dt.bfloat16, kind="ExternalInput").ap()
    if has_bqk:
        bqk_d = nc.dram_tensor("bqk", [1, 2 * H], dt.bfloat16, kind="ExternalInput").ap()
    if has_bv:
        bv_d = nc.dram_tensor("bv", [1, H], dt.bfloat16, kind="ExternalInput").ap()
    y_d = nc.dram_tensor("y", [S, H], dt.float32, kind="ExternalOutput").ap()

    EXP = mybir.ActivationFunctionType.Exp
    with tile.TileContext(nc) as tc:
        with (
            tc.tile_pool(name="consts", bufs=1) as cpool,
            tc.tile_pool(name="xt", bufs=1) as xpool,
            tc.tile_pool(name="qk", bufs=1) as qkpool,
            tc.tile_pool(name="vex", bufs=1) as vpool,
            tc.tile_pool(name="pt", bufs=1) as ppool,
            tc.tile_pool(name="maskp", bufs=1) as mpool,
            tc.tile_pool(name="outs", bufs=4) as opool,
            tc.tile_pool(name="pcps", bufs=2, space="PSUM") as pcpool,   # proj + ctx
            tc.tile_pool(name="wps", bufs=2, space="PSUM") as wpsum,     # mm1 (2 banks ea)
            tc.tile_pool(name="vps", bufs=2, space="PSUM") as vpsum,     # v proj
        ):
            # ---- constants; weights/mask ride the gpsimd ring so the slabs
            # are the sync ring's only early DMAs (each HWDGE DMA's completion
            # chain serializes its ring) ----
            wqk_sb = cpool.tile([P, NE, 2 * H], dt.bfloat16)
            nc.gpsimd.dma_start(wqk_sb[:], wqk_d.rearrange("p (c h) -> p c h", c=NE))
            wv_sb = cpool.tile([P, NE, H], dt.bfloat16)
            nc.gpsimd.dma_start(wv_sb[:], wv_d.rearrange("p (c h) -> p c h", c=NE))
            mask_all = mpool.tile([P, n_mb * P], dt.bfloat16, name="mask_all")
            nc.gpsimd.dma_start(mask_all[:], mb_d[:])
            mask_tiles = {}
            for (j, i, slot) in mask_items:
                mask_tiles[(j, i)] = mask_all[:, slot * P:(slot + 1) * P]
            if has_bqk:
                bqk_sb = cpool.tile([1, 2 * H], dt.bfloat16)
                nc.gpsimd.dma_start(bqk_sb[:], bqk_d[:])
            if has_bv:
                bv_sb = cpool.tile([1, H], dt.bfloat16)
                nc.gpsimd.dma_start(bv_sb[:], bv_d[:])

            zeros_sb = cpool.tile([P, SB], dt.bfloat16)
            nc.vector.memset(zeros_sb[:], 0.0)
            ones_sb = cpool.tile([1, SB], dt.bfloat16)
            nc.vector.memset(ones_sb[:], 1.0)
            warm_sb = cpool.tile([1, 2], dt.float32)
            nc.scalar.activation(warm_sb[:], ones_sb[0:1, 0:2], EXP, scale=0.125)

            # ---- x^T slabs: one contiguous DMA per slab ----
            xqt = xpool.tile([P, NE * S], dt.bfloat16, name="xq")
            for k in range(NSL):
                a, b = NE * SLAB_S0[k], NE * (SLAB_S0[k] + SLAB_W[k])
                nc.sync.dma_start(xqt[:, a:b], xq_d[:, a:b])

            def xq_cols(k, c, lo, hi):
                """View of x^T chunk c, columns [lo, hi) (must lie in slab k)."""
                s0, w = SLAB_S0[k], SLAB_W[k]
                off = NE * s0 + c * w + (lo - s0)
                return xqt[:, off:off + (hi - lo)]

            # ---- PE warm-up: contentless matmuls to release the HAM gate ----
            warm_ps = [wpsum.tile([P, 2 * SB], dt.float32, tag="wps", name="wmps")
                       for _ in range(2)]
            for n in range(WARMUP_MMS):
                nc.tensor.matmul(warm_ps[n % 2][:, 0:SB], zeros_sb[:, 0:P],
                                 zeros_sb[:], start=True, stop=True)

            # ---- persistent SBUF tensors ----
            qkT_sb = qkpool.tile([P, S], dt.bfloat16)      # q^T rows 0-63, k^T 64-127
            kT_sb = qkpool.tile([64, S], dt.bfloat16)      # k^T moved to partitions 0-63
            vext = [vpool.tile([P, H + 1], dt.bfloat16, tag=f"vx{j}", name=f"vx{j}")
                    for j in range(NS)]
            pt = ppool.tile([P, NS, S], dt.bfloat16, name="pt_all")  # exp(scores^T)

            y_t = y_d.rearrange("(i p) h -> p i h", p=P)

            def emit_proj(k):
                """q/k projection for slab k (in <=SB sub-blocks) -> qkT_sb."""
                s0, w = SLAB_S0[k], SLAB_W[k]
                off = 0
                while off < w:
                    sw = min(SB, w - off)
                    lo = s0 + off
                    ps = pcpool.tile([P, SB], dt.float32, tag="pc", name=f"qkps{k}")
                    for c in range(NE):
                        nc.tensor.matmul(ps[:, 0:sw], wqk_sb[:, c, :],
                                         xq_cols(k, c, lo, lo + sw),
                                         start=(c == 0),
                                         stop=(not has_bqk and c == NE - 1))
                    if has_bqk:
                        nc.tensor.matmul(ps[:, 0:sw], bqk_sb[:], ones_sb[:, 0:sw],
                                         start=False, stop=True)
                    nc.vector.tensor_copy(qkT_sb[:, lo:lo + sw], ps[:, 0:sw])
                    off += sw
                # k^T partition fixup: rows 64-127 -> 0-63 (SBUF->SBUF DMA)
                nc.gpsimd.dma_start(kT_sb[:, s0:s0 + w], qkT_sb[64:128, s0:s0 + w])

            def emit_v(j):
                vt = vext[j]
                nc.vector.memset(vt[:, H:H + 1], 1.0)
                pv = vpsum.tile([P, H + 1], dt.float32, tag="vps", name=f"pv{j}")
                k = next(kk for kk in range(NSL)
                         if SLAB_S0[kk] <= j * P < SLAB_S0[kk] + SLAB_W[kk])
                for c in range(NE):
                    nc.tensor.matmul(
                        pv[:, 0:H], xq_cols(k, c, j * P, (j + 1) * P), wv_sb[:, c, :],
                        start=(c == 0), stop=(not has_bv and c == NE - 1))
                if has_bv:
                    nc.tensor.matmul(pv[:, 0:H], ones_sb[:, 0:P], bv_sb[:],
                                     start=False, stop=True)
                nc.vector.tensor_copy(vt[:, 0:H], pv[:, 0:H])

            def mm1_units(k):
                """mm1 column units for slab k, sub-range-major so units of
                adjacent t-tiles with identical columns pair up for exp."""
                s0, w = SLAB_S0[k], SLAB_W[k]
                units = []
                off = 0
                while off < w:
                    sw = min(SB, w - off)
                    a0, b0 = s0 + off, s0 + off + sw
                    for j in range(NS):
                        if ranges[j] is None:
                            continue
                        lo, hi = ranges[j]
                        a, b = max(lo, a0), min(hi, b0)
                        if a < b:
                            units.append((j, a, b - a))
                    off += sw
                return units

            def emit_mm1_pair(units):
                """Emit up to two mm1 units into one 2-bank wps tile + exp."""
                ps = wpsum.tile([P, 2 * SB], dt.float32, tag="wps", name="wps")
                for n, (j, off, w) in enumerate(units):
                    nc.tensor.matmul(
                        ps[:, n * SB:n * SB + w],
                        kT_sb[:, j * P:(j + 1) * P],
                        qkT_sb[0:64, off:off + w],
                        start=True, stop=True, tile_position=(0, 0))
                if (len(units) == 2 and units[0][2] == units[1][2]
                        and units[0][1] == units[1][1]
                        and units[1][0] == units[0][0] + 1):
                    j, off, w = units[0]
                    if w == SB:
                        nc.scalar.activation(pt[:, j:j + 2, off:off + w],
                                             ps[:], EXP, scale=0.125)
                    else:
                        nc.scalar.activation(
                            pt[:, j:j + 2, off:off + w],
                            ps.rearrange("p (n s) -> p n s", n=2)[:, :, 0:w],
                            EXP, scale=0.125)
                else:
                    for n, (j, off, w) in enumerate(units):
                        nc.scalar.activation(pt[:, j, off:off + w],
                                             ps[:, n * SB:n * SB + w],
                                             EXP, scale=0.125)

            def emit_mask_slab(k):
                """Mask-multiply every mixed block whose s-columns lie in slab k."""
                s0, w = SLAB_S0[k], SLAB_W[k]
                for (j, i), mtile in mask_tiles.items():
                    if s0 <= i * P < s0 + w:
                        sl = pt[:, j, i * P:(i + 1) * P]
                        nc.vector.tensor_mul(sl, sl, mtile)

            started = {}   # ctx-bank key (slab idx) -> first matmul emitted

            def slab_tiles(k):
                return range(SLAB_S0[k] // P, (SLAB_S0[k] + SLAB_W[k]) // P)

            def mm2_acc(i, js, pc, key, stop):
                sl = pc[:, i - SLAB_S0[key] // P, :]
                for n, j in enumerate(js):
                    st = not started.get(key, False)
                    started[key] = True
                    nc.tensor.matmul(
                        sl, pt[:, j, i * P:(i + 1) * P], vext[j][:],
                        start=st, stop=(stop and n == len(js) - 1))

            def mm2_norm_group(k, pc):
                """Normalize all tiles of slab k: one strided reciprocal,
                per-tile scale, one grouped store."""
                tiles = list(slab_tiles(k))
                nt = len(tiles)
                obg = opool.tile([P, nt, H], dt.float32, tag=f"outg{k % 2}",
                                 name=f"obg{k}")
                rc = opool.tile([P, nt], dt.float32, tag="recipg", name=f"rcg{k}")
                nc.vector.reciprocal(rc[:], pc[:, :, H])
                for t, i in enumerate(tiles):
                    if not mm2[i]:
                        nc.vector.memset(obg[:, t, :], 0.0)
                    else:
                        nc.vector.tensor_scalar_mul(obg[:, t, :],
                                                    pc[:, t, 0:H],
                                                    rc[:, t:t + 1])
                nc.sync.dma_start(y_t[:, tiles[0]:tiles[0] + nt, :], obg[:])

            def emit_fillers(n):
                """Zero-dependency matmuls: bridge input-DMA stalls so the PE
                HAM clock gate stays released (a >2us idle window re-throttles
                the PE to 1.2 GHz for the next ~3.4us of work)."""
                for _ in range(n):
                    fps = wpsum.tile([P, 2 * SB], dt.float32, tag="wps",
                                     name="fill")
                    nc.tensor.matmul(fps[:, 0:SB], zeros_sb[:, 0:P],
                                     zeros_sb[:], start=True, stop=True)

            # ---- s-major streaming over slabs ----
            pending = []   # deferred mm2/norm callables from the previous slab
            FILLERS = {0: 3, 1: 7, 2: 3}

            for k in range(NSL):
                emit_proj(k)
                units = mm1_units(k)
                tiles = list(slab_tiles(k))
                n_nonlocal = sum(1 for (j, off, w) in units if j < tiles[0])
                last = (k == NSL - 1)
                work = [(lambda j=j: emit_v(j)) for j in tiles]
                work += pending
                pending = []
                late = []
                pc = pcpool.tile([P, len(tiles), H + 1], dt.float32, tag="pc",
                                 name=f"ctx{k}")
                if last:
                    jcut = tiles[0]
                    for i in tiles:
                        js1 = tuple(j for j in mm2[i]
                                    if j < jcut and (j, i) not in mask_tiles)
                        if js1:
                            late.append((lambda i=i, js1=js1, pc=pc, k=k:
                                         mm2_acc(i, js1, pc, k, stop=False)))
                ui = wi = li = 0
                while ui < len(units) or wi < len(work) or li < len(late):
                    if ui < len(units):
                        emit_mm1_pair(units[ui:ui + 2])
                        ui += 2
                    if wi < len(work):
                        work[wi]()
                        wi += 1
                    if li < len(late) and ui >= n_nonlocal:
                        late[li]()
                        li += 1
                emit_mask_slab(k)
                if last:
                    for i in tiles:
                        js2 = tuple(j for j in mm2[i]
                                    if j >= jcut or (j, i) in mask_tiles)
                        mm2_acc(i, js2, pc, k, stop=True)
                    mm2_norm_group(k, pc)
                else:
                    for i in tiles:
                        pending.append((lambda i=i, pc=pc, k=k:
                                        mm2_acc(i, mm2[i], pc, k, stop=True)))
                    pending.append((lambda k=k, pc=pc: mm2_norm_group(k, pc)))
                emit_fillers(FILLERS.get(k, 0))
            for fn_ in pending:
                fn_()

    nc.compile()
    return nc


def kernel(x, mask, Wq, bq, Wk, bk, Wv, bv, _trace=False, _trace_kwargs=None):
    global last_results
    x = np.asarray(x, dtype=np.float32)
    ranges, mask_items, n_uniq, mm2, maskT = _plan_from_mask(mask)

    has_bqk = bool(np.any(bq)) or bool(np.any(bk))
    has_bv = bool(np.any(bv))
    key = (tuple(ranges), mask_items, n_uniq, tuple(mm2), has_bqk, has_bv)
    nc = _cache.get(key)
    if nc is None:
        nc = _build_nc(ranges, mask_items, n_uniq, mm2, has_bqk, has_bv)
        _cache[key] = nc

    wqk = np.concatenate([np.asarray(Wq), np.asarray(Wk)], axis=1)
    wqk = np.ascontiguousarray(
        wqk.reshape(NE, P, 2 * H).transpose(1, 0, 2)).reshape(P, NE * 2 * H).astype(BF16)
    wv = np.ascontiguousarray(
        np.asarray(Wv).reshape(NE, P, H).transpose(1, 0, 2)).reshape(P, NE * H).astype(BF16)
    bqk = np.concatenate([np.asarray(bq), np.asarray(bk)])[None, :].astype(BF16)
    bvv = np.asarray(bv)[None, :].astype(BF16)

    in_maps = []
    for b in range(B):
        # slab-major packing: [p][slab k: chunk-major columns]
        xT3 = x[b].T.astype(BF16).reshape(NE, P, S)
        xqb = np.concatenate(
            [np.ascontiguousarray(
                xT3[:, :, s0:s0 + w].transpose(1, 0, 2)).reshape(P, NE * w)
             for s0, w in zip(SLAB_S0, SLAB_W)], axis=1)
        if mask_items:
            by_slot = {}
            for (j, i, slot) in mask_items:
                by_slot.setdefault(slot, (j, i))
            mb = np.concatenate([
                maskT[b, j * P:(j + 1) * P, i * P:(i + 1) * P]
                for slot, (j, i) in sorted(by_slot.items())], axis=1).astype(BF16)
        else:
            mb = np.zeros((P, P), dtype=BF16)
        im = {"xq": xqb, "wqk": wqk, "wv": wv, "maskb": mb}
        if has_bqk:
            im["bqk"] = bqk
        if has_bv:
            im["bv"] = bvv
        in_maps.append(im)

    res = run_bass_kernel_spmd(
        nc, in_maps, core_ids=list(range(B)),
        trace=_trace, **(_trace_kwargs or {}))
    last_results = res
    return np.stack([res.results[b]["y"] for b in range(B)])


# revision 32
# speedup vs baseline: 1.0434x; 1.0434x over previous
"""Trainium2 Bass kernel for a single-head BERT attention (B=8, S=2048, E=1024, H=64).

Sharding: data-parallel over batch — one batch element per NeuronCore (8 cores).
Weights replicated. No collectives.

Layout (all matmuls bf16 with fp32 PSUM accumulation):
  qkT  = [Wq|Wk]^T-projection: [128, S] (rows 0-63 = q^T, 64-127 = k^T)
  v    = natural [S, H] per 128-row tile + appended ones column (softmax denom)
  ptT  = exp(scoresT / 8) per t-tile, multiplied by the mask on mixed blocks
  ctx  = ptT.T @ [v|1], rows normalized by 1/denom

Schedule is s-major streaming: x^T arrives as 4 contiguous 1 MB s-slabs
(host pre-packs the DRAM layout so each slab is one descriptor per
partition). Per slab: project q/k for those s columns, project v for the
slab's t-tiles, run all mm1 columns that end in the slab, exp them, then
finish the slab's four output tiles (mm2 + normalize + per-tile DMA out).
Dummy warm-up matmuls run during the runtime preamble/input DMA so the PE
HAM clock gate is released before real work, and the kernel keeps PE/ACT
co-busy throughout so it never re-throttles.
"""

import numpy as np
import ml_dtypes

import concourse.bass as bass  # noqa: F401  (import registers bass machinery)
import concourse.bacc as bacc
import concourse.mybir as mybir
import concourse.tile as tile
from concourse.bass_utils import run_bass_kernel_spmd

BF16 = ml_dtypes.bfloat16
B, S, E, H = 8, 2048, 1024, 64
P = 128          # partitions / tile edge
NS = S // P      # 16 seq tiles
NE = E // P      # 8 embed chunks
SLAB = 512       # s-major slab width (one PSUM bank of fp32)
NSL = S // SLAB  # 4 slabs
TPS = SLAB // P  # 4 t-tiles per slab
WARMUP_MMS = 13  # dummy matmuls to release the HAM clock gate early

_cache: dict = {}
last_results = None  # BassKernelResults of the most recent run (for test harness)


def _plan_from_mask(mask: np.ndarray):
    """Derive the static block plan from the actual mask input.

    Returns (ranges, mask_items, n_uniq, mm2_lists, maskT):
      ranges[j]     = (lo, hi) element range of s that t-tile j must compute (or None)
      mask_items    = ((j, i, slot)) 128x128 blocks needing an elementwise mask
                      multiply; slot indexes the deduped unique-block upload
      mm2_lists[i]  = t-tiles contributing to output s-tile i
    Valid for every batch element simultaneously (classifications reduced over batch).
    """
    m = np.asarray(mask, dtype=bool)
    mt = np.ascontiguousarray(m.transpose(0, 2, 1))  # [B, t, s]
    blocks = mt.reshape(B, NS, P, NS, P)
    any_ = blocks.any(axis=(2, 4))   # [B, tj, si]
    all_ = blocks.all(axis=(2, 4))
    nz = any_.any(axis=0)            # not all-zero in some batch -> must compute
    allone = all_.all(axis=0)        # all-ones in every batch -> no mask needed
    mixed = nz & ~allone

    ranges = []
    for j in range(NS):
        cols = np.nonzero(nz[j])[0]
        if len(cols) == 0:
            ranges.append(None)
            continue
        ranges.append((int(cols.min()) * P, (int(cols.max()) + 1) * P))

    uniq = {}
    mask_items = []
    for j in range(NS):
        for i in range(NS):
            if not mixed[j, i]:
                continue
            key = blocks[:, j, :, i, :].tobytes()
            slot = uniq.setdefault(key, len(uniq))
            mask_items.append((j, i, slot))
    mm2 = [tuple(int(j) for j in np.nonzero(nz[:, i])[0]) for i in range(NS)]
    return ranges, tuple(mask_items), len(uniq), mm2, mt


def _build_nc(ranges, mask_items, n_uniq, mm2, has_bqk, has_bv):
    dt = mybir.dt
    n_mb = max(n_uniq, 1)
    nc = bacc.Bacc("TRN2", target_bir_lowering=False, debug=False, num_devices=8)

    # x^T packed host-side as [p, slab, chunk, s_in_slab] so one slab is a
    # single contiguous 8 KB read per partition.
    xq_d = nc.dram_tensor("xq", [P, NSL * NE * SLAB], dt.bfloat16,
                          kind="ExternalInput").ap()
    wqk_d = nc.dram_tensor("wqk", [P, NE * 2 * H], dt.bfloat16, kind="ExternalInput").ap()
    wv_d = nc.dram_tensor("wv", [P, NE * H], dt.bfloat16, kind="ExternalInput").ap()
    mb_d = nc.dram_tensor("maskb", [P, n_mb * P], dt.bfloat16, kind="ExternalInput").ap()
    if has_bqk:
        bqk_d = nc.dram_tensor("bqk", [1, 2 * H], dt.bfloat16, kind="ExternalInput").ap()
    if has_bv:
        bv_d = nc.dram_tensor("bv", [1, H], dt.bfloat16, kind="ExternalInput").ap()
    y_d = nc.dram_tensor("y", [S, H], dt.float32, kind="ExternalOutput").ap()

    EXP = mybir.ActivationFunctionType.Exp
    with tile.TileContext(nc) as tc:
        with (
            tc.tile_pool(name="consts", bufs=1) as cpool,
            tc.tile_pool(name="xt", bufs=1) as xpool,
            tc.tile_pool(name="qk", bufs=1) as qkpool,
            tc.tile_pool(name="vex", bufs=1) as vpool,
            tc.tile_pool(name="pt", bufs=1) as ppool,
            tc.tile_pool(name="maskp", bufs=1) as mpool,
            tc.tile_pool(name="outs", bufs=4) as opool,
            tc.tile_pool(name="pcps", bufs=2, space="PSUM") as pcpool,   # proj + ctx
            tc.tile_pool(name="wps", bufs=2, space="PSUM") as wpsum,     # mm1 (2 banks ea)
            tc.tile_pool(name="vps", bufs=2, space="PSUM") as vpsum,     # v proj
        ):
            # ---- constants ----
            wqk_sb = cpool.tile([P, NE, 2 * H], dt.bfloat16)
            nc.sync.dma_start(wqk_sb[:], wqk_d.rearrange("p (c h) -> p c h", c=NE))
            zeros_sb = cpool.tile([P, SLAB], dt.bfloat16)
            nc.vector.memset(zeros_sb[:], 0.0)
            ones_sb = cpool.tile([1, SLAB], dt.bfloat16)
            nc.vector.memset(ones_sb[:], 1.0)
            warm_sb = cpool.tile([1, 2], dt.float32)
            nc.scalar.activation(warm_sb[:], ones_sb[0:1, 0:2], EXP, scale=0.125)

            # v weights + deduped mask blocks ride the gpsimd (SWDGE) queue
            wv_sb = cpool.tile([P, NE, H], dt.bfloat16)
            nc.gpsimd.dma_start(wv_sb[:], wv_d.rearrange("p (c h) -> p c h", c=NE))
            mask_all = mpool.tile([P, n_mb * P], dt.bfloat16, name="mask_all")
            nc.gpsimd.dma_start(mask_all[:], mb_d[:])
            mask_tiles = {}
            for (j, i, slot) in mask_items:
                mask_tiles[(j, i)] = mask_all[:, slot * P:(slot + 1) * P]
            if has_bqk:
                bqk_sb = cpool.tile([1, 2 * H], dt.bfloat16)
                nc.gpsimd.dma_start(bqk_sb[:], bqk_d[:])
            if has_bv:
                bv_sb = cpool.tile([1, H], dt.bfloat16)
                nc.gpsimd.dma_start(bv_sb[:], bv_d[:])

            # ---- x^T slabs: one DMA per slab, issued in consumption order ----
            xq = xpool.tile([P, NSL, NE, SLAB], dt.bfloat16, name="xq")
            xqg = xq_d.rearrange("p (k c s) -> p k c s", k=NSL, c=NE)
            for k in range(NSL):
                nc.sync.dma_start(xq[:, k], xqg[:, k])

            # ---- PE warm-up: contentless matmuls to release the HAM gate ----
            warm_ps = [wpsum.tile([P, 2 * SLAB], dt.float32, tag="wps", name="wmps")
                       for _ in range(2)]
            for n in range(WARMUP_MMS):
                nc.tensor.matmul(warm_ps[n % 2][:, 0:SLAB], zeros_sb[:, 0:P],
                                 zeros_sb[:], start=True, stop=True)

            # ---- persistent SBUF tensors ----
            qkT_sb = qkpool.tile([P, S], dt.bfloat16)      # q^T rows 0-63, k^T 64-127
            kT_sb = qkpool.tile([64, S], dt.bfloat16)      # k^T moved to partitions 0-63
            vext = [vpool.tile([P, H + 1], dt.bfloat16, tag=f"vx{j}", name=f"vx{j}")
                    for j in range(NS)]
            pt = ppool.tile([P, NS, S], dt.bfloat16, name="pt_all")  # exp(scores^T)

            y_t = y_d.rearrange("(i p) h -> p i h", p=P)

            def emit_proj(k):
                """q/k projection for s-slab k -> qkT_sb[:, k*SLAB:(k+1)*SLAB]."""
                cols = slice(k * SLAB, (k + 1) * SLAB)
                ps = pcpool.tile([P, SLAB], dt.float32, tag="pc", name=f"qkps{k}")
                for c in range(NE):
                    nc.tensor.matmul(ps[:], wqk_sb[:, c, :], xq[:, k, c, :],
                                     start=(c == 0),
                                     stop=(not has_bqk and c == NE - 1))
                if has_bqk:
                    nc.tensor.matmul(ps[:], bqk_sb[:], ones_sb[:],
                                     start=False, stop=True)
                nc.vector.tensor_copy(qkT_sb[:, cols], ps[:])
                # k^T partition fixup: rows 64-127 -> 0-63 (SBUF->SBUF DMA)
                nc.gpsimd.dma_start(kT_sb[:, cols], qkT_sb[64:128, cols])

            def emit_v(j):
                vt = vext[j]
                nc.vector.memset(vt[:, H:H + 1], 1.0)
                pv = vpsum.tile([P, H + 1], dt.float32, tag="vps", name=f"pv{j}")
                k, t = divmod(j, TPS)
                for c in range(NE):
                    nc.tensor.matmul(
                        pv[:, 0:H], xq[:, k, c, t * P:(t + 1) * P], wv_sb[:, c, :],
                        start=(c == 0), stop=(not has_bv and c == NE - 1))
                if has_bv:
                    nc.tensor.matmul(pv[:, 0:H], ones_sb[:, 0:P], bv_sb[:],
                                     start=False, stop=True)
                nc.vector.tensor_copy(vt[:, 0:H], pv[:, 0:H])

            def mm1_units(k):
                """mm1 column units for slab k: [(j, off, w), ...]."""
                units = []
                lo_s, hi_s = k * SLAB, (k + 1) * SLAB
                for j in range(NS):
                    if ranges[j] is None:
                        continue
                    lo, hi = ranges[j]
                    a, b = max(lo, lo_s), min(hi, hi_s)
                    if a < b:
                        units.append((j, a, b - a))
                return units

            def emit_mm1_pair(units):
                """Emit up to two mm1 units into one 2-bank wps tile + one exp each."""
                ps = wpsum.tile([P, 2 * SLAB], dt.float32, tag="wps", name="wps")
                for n, (j, off, w) in enumerate(units):
                    nc.tensor.matmul(
                        ps[:, n * SLAB:n * SLAB + w],
                        kT_sb[:, j * P:(j + 1) * P],
                        qkT_sb[0:64, off:off + w],
                        start=True, stop=True, tile_position=(0, 0))
                if (len(units) == 2 and units[0][2] == units[1][2]
                        and units[0][1] == units[1][1]
                        and units[1][0] == units[0][0] + 1):
                    # same column range, adjacent j: single strided-exp
                    j, off, w = units[0]
                    if w == SLAB:
                        nc.scalar.activation(pt[:, j:j + 2, off:off + w],
                                             ps[:], EXP, scale=0.125)
                    else:
                        nc.scalar.activation(
                            pt[:, j:j + 2, off:off + w],
                            ps.rearrange("p (n s) -> p n s", n=2)[:, :, 0:w],
                            EXP, scale=0.125)
                else:
                    for n, (j, off, w) in enumerate(units):
                        nc.scalar.activation(pt[:, j, off:off + w],
                                             ps[:, n * SLAB:n * SLAB + w],
                                             EXP, scale=0.125)

            def emit_mask_slab(k):
                """Mask-multiply every mixed block whose s-columns lie in slab k."""
                for (j, i), mtile in mask_tiles.items():
                    if k * SLAB <= i * P < (k + 1) * SLAB:
                        sl = pt[:, j, i * P:(i + 1) * P]
                        nc.vector.tensor_mul(sl, sl, mtile)

            started = {}   # pc-bank key -> first matmul already emitted

            def mm2_acc(i, js, pc, key, stop):
                sl = pc[:, i % TPS, :]
                for n, j in enumerate(js):
                    st = not started.get(key, False)
                    started[key] = True
                    nc.tensor.matmul(
                        sl, pt[:, j, i * P:(i + 1) * P], vext[j][:],
                        start=st, stop=(stop and n == len(js) - 1))

            def mm2_fin(i, js, pc, key):
                """Final accumulation for tile i, then normalize + DMA out."""
                ob = opool.tile([P, H], dt.float32, tag="out", name=f"ob{i}")
                if not mm2[i]:
                    nc.vector.memset(ob[:], 0.0)
                else:
                    sl = pc[:, i % TPS, :]
                    if js:
                        mm2_acc(i, js, pc, key, stop=True)
                    rc = opool.tile([P, 1], dt.float32, tag="recip", name="rc")
                    nc.vector.reciprocal(rc[:], sl[:, H:H + 1])
                    nc.vector.tensor_scalar_mul(ob[:], sl[:, 0:H], rc[:])
                nc.sync.dma_start(y_t[:, i, :], ob[:])

            # ---- s-major streaming over slabs ----
            # Per slab: proj, v for its t-tiles, mm1+exp for all columns ending
            # here, mask fixups, then mm2+normalize+store of its 4 output tiles.
            # mm2 of slab k is emitted interleaved into slab k+1's stream so the
            # PE queue never heads-of-line blocks on exp completions. For the
            # last slab, mm2 contributions from earlier t-tiles are accumulated
            # as soon as the full-width exps land, leaving only the slab's own
            # t-tiles (plus normalize/store) for the tail.
            pending = []   # deferred finalize callables from the previous slab

            for k in range(NSL):
                emit_proj(k)
                units = mm1_units(k)
                n_full = sum(1 for (j, off, w) in units if j < k * TPS)
                last = (k == NSL - 1)
                work = [(lambda j=j: emit_v(j)) for j in
                        (k * TPS + t for t in range(TPS))]
                work += pending
                pending = []
                late = []
                pc = pcpool.tile([P, TPS, H + 1], dt.float32, tag="pc",
                                 name=f"ctx{k}")
                key = f"ctx{k}"
                if last:
                    jcut = k * TPS
                    for t in range(TPS):
                        i = k * TPS + t
                        js1 = tuple(j for j in mm2[i]
                                    if j < jcut and (j, i) not in mask_tiles)
                        if js1:
                            late.append((lambda i=i, js1=js1, pc=pc, key=key:
                                         mm2_acc(i, js1, pc, key, stop=False)))
                ui = wi = li = 0
                while ui < len(units) or wi < len(work) or li < len(late):
                    if ui < len(units):
                        emit_mm1_pair(units[ui:ui + 2])
                        ui += 2
                    if wi < len(work):
                        work[wi]()
                        wi += 1
                    if li < len(late) and ui >= n_full:
                        late[li]()
                        li += 1
                emit_mask_slab(k)
                for t in range(TPS):
                    i = k * TPS + t
                    if last:
                        js2 = tuple(j for j in mm2[i]
                                    if j >= k * TPS or (j, i) in mask_tiles)
                        pending.append((lambda i=i, js2=js2, pc=pc, key=key:
                                        mm2_fin(i, js2, pc, key)))
                    else:
                        pending.append((lambda i=i, pc=pc, key=key:
                                        mm2_fin(i, mm2[i], pc, key)))
            for fn_ in pending:
                fn_()

    nc.compile()
    return nc


def kernel(x, mask, Wq, bq, Wk, bk, Wv, bv, _trace=False, _trace_kwargs=None):
    global last_results
    x = np.asarray(x, dtype=np.float32)
    ranges, mask_items, n_uniq, mm2, maskT = _plan_from_mask(mask)

    has_bqk = bool(np.any(bq)) or bool(np.any(bk))
    has_bv = bool(np.any(bv))
    key = (tuple(ranges), mask_items, n_uniq, tuple(mm2), has_bqk, has_bv)
    nc = _cache.get(key)
    if nc is None:
        nc = _build_nc(ranges, mask_items, n_uniq, mm2, has_bqk, has_bv)
        _cache[key] = nc

    wqk = np.concatenate([np.asarray(Wq), np.asarray(Wk)], axis=1)
    wqk = np.ascontiguousarray(
        wqk.reshape(NE, P, 2 * H).transpose(1, 0, 2)).reshape(P, NE * 2 * H).astype(BF16)
    wv = np.ascontiguousarray(
        np.asarray(Wv).reshape(NE, P, H).transpose(1, 0, 2)).reshape(P, NE * H).astype(BF16)
    bqk = np.concatenate([np.asarray(bq), np.asarray(bk)])[None, :].astype(BF16)
    bvv = np.asarray(bv)[None, :].astype(BF16)

    in_maps = []
    for b in range(B):
        # [p, slab, chunk, s] so each slab is contiguous per partition
        xT_b = x[b].T.astype(BF16)                       # [E, S]
        xqb = np.ascontiguousarray(
            xT_b.reshape(NE, P, NSL, SLAB).transpose(1, 2, 0, 3)
        ).reshape(P, NSL * NE * SLAB)
        if mask_items:
            by_slot = {}
            for (j, i, slot) in mask_items:
                by_slot.setdefault(slot, (j, i))
            mb = np.concatenate([
                maskT[b, j * P:(j + 1) * P, i * P:(i + 1) * P]
                for slot, (j, i) in sorted(by_slot.items())], axis=1).astype(BF16)
        else:
            mb = np.zeros((P, P), dtype=BF16)
        im = {"xq": xqb, "wqk": wqk, "wv": wv, "maskb": mb}
        if has_bqk:
            im["bqk"] = bqk
        if has_bv:
            im["bv"] = bvv
        in_maps.append(im)

    res = run_bass_kernel_spmd(
        nc, in_maps, core_ids=list(range(B)),
        trace=_trace, **(_trace_kwargs or {}))
    last_results = res
    return np.stack([res.results[b]["y"] for b in range(B)])
